# revision 5
# baseline (speedup 1.0000x reference)
"""KNN retrieval kernel for Trainium2 (8 NeuronCores, Bass/Tile).

Computes cosine-similarity top-(K+1) retrieval + majority vote, matching:
    coll_n = l2norm(collection); q = l2norm(embedding)
    cos = coll_n @ q; vals, idx = top_k(cos, 11)
    neigh = idx[1:10]; probs = vals[1:10]; preds = labels[neigh]
    pred = argmax(bincount(preds)); confidence = probs[argmax(preds == pred)]

Sharding: collection rows are split across 8 cores. Each core gets a
contiguous *view* of 31360 rows (245 tiles of 128 rows): core c starts at
c*31250 - 110 (core 0 at 0), so shards overlap by 110 rows — this avoids any
host-side padding/copying of the 512MB collection; duplicate candidate rows
are deduped on the host.

Per 128-row tile the device computes (memory-bound, one pass per engine):
  - VectorE  tensor_tensor_reduce: dot[p] = sum_d x[p,d] * q[d]
  - ScalarE  activation(Square, accum_out): norm[p] = sum_d x[p,d]^2
Epilogue: cos = dot * 1/sqrt(norm + 1e-12), then two rounds of
max/max_index/match_replace give the per-partition top-16 candidate indices.
Host recomputes exact f32 cos for the 8*2048 candidates from the shipped
dot/norm buffers, merges, and applies the top-11 + vote logic.
"""

import numpy as np

N = 250000
D = 512
K = 10
NUM_CLASSES = 100
N_CORES = 8
RPC = N // N_CORES            # 31250 rows owned per core
T = 245                       # 128-row tiles per core
RP = T * 128                  # 31360 rows read per core (110 overlap)
B = 7                         # tiles per DMA superchunk (1.75 MB)
NEG_FILL = -30.0              # match_replace fill (below any cosine)

_CACHE = {}


def _build_device_program():
    import concourse.bacc as bacc
    import concourse.tile as tile
    from concourse import mybir

    f32 = mybir.dt.float32
    u32 = mybir.dt.uint32
    Alu = mybir.AluOpType
    Act = mybir.ActivationFunctionType

    nc = bacc.Bacc(
        "TRN2",
        target_bir_lowering=False,
        debug=False,
        enable_asserts=False,
        num_devices=N_CORES,
    )
    coll = nc.dram_tensor("coll", [RP, D], f32, kind="ExternalInput").ap()
    qrep = nc.dram_tensor("qrep", [128, D], f32, kind="ExternalInput").ap()
    dotb = nc.dram_tensor("dotb", [128, T], f32, kind="ExternalOutput").ap()
    normb = nc.dram_tensor("normb", [128, T], f32, kind="ExternalOutput").ap()
    cidx = nc.dram_tensor("cidx", [128, 16], u32, kind="ExternalOutput").ap()

    # [RP, D] -> [T/B, 128, B, D]: superchunk j, partition p holds rows
    # (j*B+b)*128 + p for b in [0,B)
    coll_v = coll.rearrange("(j b p) d -> j p b d", b=B, p=128)

    with tile.TileContext(nc) as tc:
        with (
            tc.tile_pool(name="sc", bufs=4) as sc_pool,
            tc.tile_pool(name="scr", bufs=2) as scr_pool,
            tc.tile_pool(name="per", bufs=1) as per,
        ):
            q_t = per.tile([128, D], f32)
            nc.sync.dma_start(q_t[:], qrep[:])
            dot_t = per.tile([128, T], f32)
            norm_t = per.tile([128, T], f32)

            for j in range(T // B):
                sc = sc_pool.tile([128, B * D], f32, tag="sc")
                sc3 = sc[:].rearrange("p (b d) -> p b d", d=D)
                nc.sync.dma_start(sc3, coll_v[j])
                for b in range(B):
                    t = j * B + b
                    a = sc[:, b * D : (b + 1) * D]
                    prod = scr_pool.tile([128, D], f32, tag="prod")
                    nc.vector.affine_mul_reduce(
                        out=prod[:],
                        accum_out=dot_t[:, t : t + 1],
                        in0=a,
                        in1=q_t[:],
                        scale=1.0,
                        bias=0.0,
                    )
                    sq = scr_pool.tile([128, D], f32, tag="sq")
                    nc.scalar.activation(
                        sq[:], a, Act.Square, accum_out=norm_t[:, t : t + 1]
                    )

            # cos = dot / sqrt(norm + 1e-12)
            eps_t = per.tile([128, 1], f32)
            nc.vector.memset(eps_t[:], 1e-12)
            sqn = per.tile([128, T], f32)
            nc.scalar.activation(sqn[:], norm_t[:], Act.Sqrt, bias=eps_t[:])
            rec = per.tile([128, T], f32)
            nc.vector.reciprocal(rec[:], sqn[:])
            cos_t = per.tile([128, T], f32)
            nc.vector.tensor_mul(cos_t[:], dot_t[:], rec[:])

            # per-partition top-16 indices (two rounds of top-8)
            mx1 = per.tile([128, 8], f32)
            nc.vector.max(out=mx1[:], in_=cos_t[:])
            ci_t = per.tile([128, 16], u32)
            nc.vector.max_index(ci_t[:, 0:8], mx1[:], cos_t[:])
            cos2 = per.tile([128, T], f32)
            nc.vector.match_replace(
                out=cos2[:], in_to_replace=mx1[:], in_values=cos_t[:],
                imm_value=NEG_FILL,
            )
            mx2 = per.tile([128, 8], f32)
            nc.vector.max(out=mx2[:], in_=cos2[:])
            nc.vector.max_index(ci_t[:, 8:16], mx2[:], cos2[:])

            nc.sync.dma_start(dotb[:], dot_t[:])
            nc.sync.dma_start(normb[:], norm_t[:])
            nc.sync.dma_start(cidx[:], ci_t[:])

    nc.compile()
    return nc


def _get_program():
    if "nc" not in _CACHE:
        _CACHE["nc"] = _build_device_program()
    return _CACHE["nc"]


def _core_starts():
    # core c reads rows [start, start + RP); overlap of RP - RPC = 110 rows
    return [0] + [c * RPC - (RP - RPC) for c in range(1, N_CORES)]


def build_in_maps(embedding, collection):
    emb = np.asarray(embedding, dtype=np.float32)
    coll = np.ascontiguousarray(np.asarray(collection, dtype=np.float32))
    q = emb / np.sqrt(np.sum(emb * emb, dtype=np.float32) + np.float32(1e-12))
    qrep = np.ascontiguousarray(np.broadcast_to(q[None, :], (128, D)))
    in_maps = []
    for start in _core_starts():
        in_maps.append({"coll": coll[start : start + RP], "qrep": qrep})
    return in_maps


def run_device(in_maps, trace=False, **kwargs):
    from concourse.bass_utils import run_bass_kernel_spmd

    nc = _get_program()
    return run_bass_kernel_spmd(
        nc, in_maps, list(range(N_CORES)), trace=trace, **kwargs
    )


def postprocess(results, labels_int):
    labels = np.asarray(labels_int)
    p = np.arange(128, dtype=np.int64)[:, None]
    all_rows, all_vals = [], []
    for c, start in enumerate(_core_starts()):
        dot = results[c]["dotb"]
        norm = results[c]["normb"]
        ci = results[c]["cidx"].astype(np.int64)
        cos = dot / np.sqrt(norm + np.float32(1e-12))
        rloc = ci * 128 + p                      # [128, 16] shard-local rows
        all_rows.append((rloc + start).ravel())
        all_vals.append(cos[p, ci].ravel())
    rows = np.concatenate(all_rows)
    vals = np.concatenate(all_vals)
    # dedupe overlap rows (identical values; keep one)
    rows, uidx = np.unique(rows, return_index=True)
    vals = vals[uidx]
    # global top-(K+1), ties broken by smaller row index (matches lax.top_k)
    order = np.lexsort((rows, -vals))[: K + 1]
    vals11 = vals[order].astype(np.float32)
    idx11 = rows[order]

    neigh = idx11[1:K]
    probs = vals11[1:K]
    preds = labels[neigh].astype(np.int64)
    counts = np.bincount(preds, minlength=NUM_CLASSES)
    pred_int = int(np.argmax(counts))
    conf_pos = int(np.argmax(preds == pred_int))
    confidence = probs[conf_pos]
    return (
        vals11,
        np.int32(pred_int),
        np.float32(confidence),
    )


def kernel(embedding, collection, labels_int):
    in_maps = build_in_maps(embedding, collection)
    res = run_device(in_maps)
    return postprocess(res.results, labels_int)


# revision 10
# speedup vs baseline: 1.1407x; 1.1407x over previous
"""KNN retrieval kernel for Trainium2 (8 NeuronCores, Bass/Tile).

Computes cosine-similarity top-(K+1) retrieval + majority vote, matching:
    coll_n = l2norm(collection); q = l2norm(embedding)
    cos = coll_n @ q; vals, idx = top_k(cos, 11)
    neigh = idx[1:10]; probs = vals[1:10]; preds = labels[neigh]
    pred = argmax(bincount(preds)); confidence = probs[argmax(preds == pred)]

Sharding: collection rows are split across 8 cores. Each core gets a
contiguous *view* of 31360 rows (245 tiles of 128 rows): core c starts at
c*31250 - 110 (core 0 at 0), so shards overlap by 110 rows — this avoids any
host-side padding/copying of the 512MB collection; duplicate candidate rows
are deduped on the host.

Per 128-row tile the device computes (memory-bound, one pass per engine):
  - VectorE  tensor_tensor_reduce: dot[p] = sum_d x[p,d] * q[d]
  - ScalarE  activation(Square, accum_out): norm[p] = sum_d x[p,d]^2
Epilogue: cos = dot * 1/sqrt(norm + 1e-12), then two rounds of
max/max_index/match_replace give the per-partition top-16 candidate indices.
Host recomputes exact f32 cos for the 8*2048 candidates from the shipped
dot/norm buffers, merges, and applies the top-11 + vote logic.
"""

import numpy as np

N = 250000
D = 512
K = 10
NUM_CLASSES = 100
N_CORES = 8
RPC = N // N_CORES            # 31250 rows owned per core
T = 245                       # 128-row tiles per core
RP = T * 128                  # 31360 rows read per core (110 overlap)
B = 7                         # tiles per DMA superchunk (1.75 MB)
NEG_FILL = -30.0              # match_replace fill (below any cosine)
NORM_DVE_EVERY = 8            # every 8th tile's norm on VectorE (rebalance)
NORM_DVE_PHASE = 3

_CACHE = {}


def _build_device_program():
    import concourse.bacc as bacc
    import concourse.tile as tile
    from concourse import mybir

    f32 = mybir.dt.float32
    u32 = mybir.dt.uint32
    Alu = mybir.AluOpType
    Act = mybir.ActivationFunctionType

    nc = bacc.Bacc(
        "TRN2",
        target_bir_lowering=False,
        debug=False,
        enable_asserts=False,
        num_devices=N_CORES,
    )
    coll = nc.dram_tensor("coll", [RP, D], f32, kind="ExternalInput").ap()
    qrep = nc.dram_tensor("qrep", [128, D], f32, kind="ExternalInput").ap()
    dotb = nc.dram_tensor("dotb", [128, T], f32, kind="ExternalOutput").ap()
    normb = nc.dram_tensor("normb", [128, T], f32, kind="ExternalOutput").ap()
    cidx = nc.dram_tensor("cidx", [128, 16], u32, kind="ExternalOutput").ap()

    # [RP, D] -> [T/B, 128, B*D]: superchunk j, partition p holds B
    # consecutive rows j*128*B + p*B + b, i.e. one contiguous 14KB DMA
    # descriptor per partition.
    coll_v = coll.rearrange("(j p b) d -> j p (b d)", b=B, p=128)

    with tile.TileContext(nc) as tc:
        with (
            tc.tile_pool(name="sc", bufs=6) as sc_pool,
            tc.tile_pool(name="scr", bufs=2) as scr_pool,
            tc.tile_pool(name="per", bufs=1) as per,
        ):
            q_t = per.tile([128, D], f32)
            nc.sync.dma_start(q_t[:], qrep[:])
            dot_t = per.tile([128, T], f32)
            norm_t = per.tile([128, T], f32)

            for j in range(T // B):
                sc = sc_pool.tile([128, B * D], f32, tag="sc")
                nc.sync.dma_start(sc[:], coll_v[j])
                for b in range(B):
                    t = j * B + b
                    a = sc[:, b * D : (b + 1) * D]
                    prod = scr_pool.tile([128, D], f32, tag="prod")
                    nc.vector.affine_mul_reduce(
                        out=prod[:],
                        accum_out=dot_t[:, t : t + 1],
                        in0=a,
                        in1=q_t[:],
                        scale=1.0,
                        bias=0.0,
                    )
                    sq = scr_pool.tile([128, D], f32, tag="sq")
                    if t % NORM_DVE_EVERY == NORM_DVE_PHASE:
                        # rebalance: ~1/8 of the norms on VectorE
                        nc.vector.affine_mul_reduce(
                            out=sq[:],
                            accum_out=norm_t[:, t : t + 1],
                            in0=a,
                            in1=a,
                            scale=1.0,
                            bias=0.0,
                        )
                    else:
                        nc.scalar.activation(
                            sq[:], a, Act.Square, accum_out=norm_t[:, t : t + 1]
                        )

            # cos = dot / sqrt(norm + 1e-12)
            eps_t = per.tile([128, 1], f32)
            nc.vector.memset(eps_t[:], 1e-12)
            sqn = per.tile([128, T], f32)
            nc.scalar.activation(sqn[:], norm_t[:], Act.Sqrt, bias=eps_t[:])
            rec = per.tile([128, T], f32)
            nc.vector.reciprocal(rec[:], sqn[:])
            cos_t = per.tile([128, T], f32)
            nc.vector.tensor_mul(cos_t[:], dot_t[:], rec[:])

            # per-partition top-16 indices (two rounds of top-8)
            mx1 = per.tile([128, 8], f32)
            nc.vector.max(out=mx1[:], in_=cos_t[:])
            ci_t = per.tile([128, 16], u32)
            nc.vector.max_index(ci_t[:, 0:8], mx1[:], cos_t[:])
            cos2 = per.tile([128, T], f32)
            nc.vector.match_replace(
                out=cos2[:], in_to_replace=mx1[:], in_values=cos_t[:],
                imm_value=NEG_FILL,
            )
            mx2 = per.tile([128, 8], f32)
            nc.vector.max(out=mx2[:], in_=cos2[:])
            nc.vector.max_index(ci_t[:, 8:16], mx2[:], cos2[:])

            nc.sync.dma_start(dotb[:], dot_t[:])
            nc.sync.dma_start(normb[:], norm_t[:])
            nc.sync.dma_start(cidx[:], ci_t[:])

    nc.compile()
    return nc


def _get_program():
    if "nc" not in _CACHE:
        _CACHE["nc"] = _build_device_program()
    return _CACHE["nc"]


def _core_starts():
    # core c reads rows [start, start + RP); overlap of RP - RPC = 110 rows
    return [0] + [c * RPC - (RP - RPC) for c in range(1, N_CORES)]


def build_in_maps(embedding, collection):
    emb = np.asarray(embedding, dtype=np.float32)
    coll = np.ascontiguousarray(np.asarray(collection, dtype=np.float32))
    q = emb / np.sqrt(np.sum(emb * emb, dtype=np.float32) + np.float32(1e-12))
    qrep = np.ascontiguousarray(np.broadcast_to(q[None, :], (128, D)))
    in_maps = []
    for start in _core_starts():
        in_maps.append({"coll": coll[start : start + RP], "qrep": qrep})
    return in_maps


def run_device(in_maps, trace=False, **kwargs):
    from concourse.bass_utils import run_bass_kernel_spmd

    nc = _get_program()
    return run_bass_kernel_spmd(
        nc, in_maps, list(range(N_CORES)), trace=trace, **kwargs
    )


def postprocess(results, labels_int):
    labels = np.asarray(labels_int)
    p = np.arange(128, dtype=np.int64)[:, None]
    all_rows, all_vals = [], []
    for c, start in enumerate(_core_starts()):
        dot = results[c]["dotb"]
        norm = results[c]["normb"]
        ci = results[c]["cidx"].astype(np.int64)
        cos = dot / np.sqrt(norm + np.float32(1e-12))
        # tile t = (j, b) at partition p holds shard row j*128*B + p*B + b
        rloc = (ci // B) * (128 * B) + p * B + (ci % B)
        all_rows.append((rloc + start).ravel())
        all_vals.append(cos[p, ci].ravel())
    rows = np.concatenate(all_rows)
    vals = np.concatenate(all_vals)
    # dedupe overlap rows (identical values; keep one)
    rows, uidx = np.unique(rows, return_index=True)
    vals = vals[uidx]
    # global top-(K+1), ties broken by smaller row index (matches lax.top_k)
    order = np.lexsort((rows, -vals))[: K + 1]
    vals11 = vals[order].astype(np.float32)
    idx11 = rows[order]

    neigh = idx11[1:K]
    probs = vals11[1:K]
    preds = labels[neigh].astype(np.int64)
    counts = np.bincount(preds, minlength=NUM_CLASSES)
    pred_int = int(np.argmax(counts))
    conf_pos = int(np.argmax(preds == pred_int))
    confidence = probs[conf_pos]
    return (
        vals11,
        np.int32(pred_int),
        np.float32(confidence),
    )


def kernel(embedding, collection, labels_int):
    in_maps = build_in_maps(embedding, collection)
    res = run_device(in_maps)
    return postprocess(res.results, labels_int)


# revision 14
# speedup vs baseline: 1.1647x; 1.0210x over previous
"""KNN retrieval kernel for Trainium2 (8 NeuronCores, Bass/Tile).

Computes cosine-similarity top-(K+1) retrieval + majority vote, matching:
    coll_n = l2norm(collection); q = l2norm(embedding)
    cos = coll_n @ q; vals, idx = top_k(cos, 11)
    neigh = idx[1:10]; probs = vals[1:10]; preds = labels[neigh]
    pred = argmax(bincount(preds)); confidence = probs[argmax(preds == pred)]

Sharding: collection rows are split across 8 cores. Each core gets a
contiguous *view* of 31360 rows (245 tiles of 128 rows): core c starts at
c*31250 - 110 (core 0 at 0), so shards overlap by 110 rows — this avoids any
host-side padding/copying of the 512MB collection; duplicate candidate rows
are deduped on the host.

Per 128-row tile the device computes (memory-bound, one pass per engine):
  - VectorE  tensor_tensor_reduce: dot[p] = sum_d x[p,d] * q[d]
  - ScalarE  activation(Square, accum_out): norm[p] = sum_d x[p,d]^2
Epilogue: cos = dot * 1/sqrt(norm + 1e-12), then two rounds of
max/max_index/match_replace give the per-partition top-16 candidate indices.
Host recomputes exact f32 cos for the 8*2048 candidates from the shipped
dot/norm buffers, merges, and applies the top-11 + vote logic.
"""

import numpy as np

N = 250000
D = 512
K = 10
NUM_CLASSES = 100
N_CORES = 8
RPC = N // N_CORES            # 31250 rows owned per core
T = 245                       # 128-row tiles per core
RP = T * 128                  # 31360 rows read per core (110 overlap)
B = 7                         # tiles per DMA superchunk (1.75 MB)
NEG_FILL = -30.0              # match_replace fill (below any cosine)
NORM_DVE_EVERY = 8            # every 8th tile's norm on VectorE (rebalance)
NORM_DVE_PHASE = 3

_CACHE = {}


def _build_device_program():
    import concourse.bacc as bacc
    import concourse.tile as tile
    from concourse import mybir

    f32 = mybir.dt.float32
    u32 = mybir.dt.uint32
    Alu = mybir.AluOpType
    Act = mybir.ActivationFunctionType

    nc = bacc.Bacc(
        "TRN2",
        target_bir_lowering=False,
        debug=False,
        enable_asserts=False,
        num_devices=N_CORES,
    )
    coll = nc.dram_tensor("coll", [RP, D], f32, kind="ExternalInput").ap()
    qrep = nc.dram_tensor("qrep", [128, D], f32, kind="ExternalInput").ap()
    dotb = nc.dram_tensor("dotb", [128, T], f32, kind="ExternalOutput").ap()
    normb = nc.dram_tensor("normb", [128, T], f32, kind="ExternalOutput").ap()
    cidx = nc.dram_tensor("cidx", [128, 16], u32, kind="ExternalOutput").ap()

    # [RP, D] -> [T/B, 128, B*D]: superchunk j, partition p holds B
    # consecutive rows j*128*B + p*B + b, i.e. one contiguous 14KB DMA
    # descriptor per partition.
    coll_v = coll.rearrange("(j p b) d -> j p (b d)", b=B, p=128)

    with tile.TileContext(nc) as tc:
        with (
            tc.tile_pool(name="sc", bufs=6) as sc_pool,
            tc.tile_pool(name="scr", bufs=2) as scr_pool,
            tc.tile_pool(name="sqp", bufs=2, space="PSUM") as sq_pool,
            tc.tile_pool(name="per", bufs=1) as per,
        ):
            q_t = per.tile([128, D], f32)
            nc.sync.dma_start(q_t[:], qrep[:])
            dot_t = per.tile([128, T], f32)
            norm_t = per.tile([128, T], f32)

            for j in range(T // B):
                sc = sc_pool.tile([128, B * D], f32, tag="sc")
                if j == 0:
                    # split the first superchunk so compute starts sooner
                    for b in range(B):
                        nc.sync.dma_start(
                            sc[:, b * D : (b + 1) * D],
                            coll_v[j][:, b * D : (b + 1) * D],
                        )
                else:
                    nc.sync.dma_start(sc[:], coll_v[j])
                for b in range(B):
                    t = j * B + b
                    a = sc[:, b * D : (b + 1) * D]
                    prod = scr_pool.tile([128, D], f32, tag="prod")
                    nc.vector.affine_mul_reduce(
                        out=prod[:],
                        accum_out=dot_t[:, t : t + 1],
                        in0=a,
                        in1=q_t[:],
                        scale=1.0,
                        bias=0.0,
                    )
                    if t % NORM_DVE_EVERY == NORM_DVE_PHASE:
                        sq = scr_pool.tile([128, D], f32, tag="sq")
                        # rebalance: ~1/8 of the norms on VectorE
                        nc.vector.affine_mul_reduce(
                            out=sq[:],
                            accum_out=norm_t[:, t : t + 1],
                            in0=a,
                            in1=a,
                            scale=1.0,
                            bias=0.0,
                        )
                    else:
                        sqp = sq_pool.tile([128, D], f32, tag="sqp")
                        nc.scalar.activation(
                            sqp[:], a, Act.Square, accum_out=norm_t[:, t : t + 1]
                        )

            # cos = dot / sqrt(norm + 1e-12)
            eps_t = per.tile([128, 1], f32)
            nc.vector.memset(eps_t[:], 1e-12)
            sqn = per.tile([128, T], f32)
            nc.scalar.activation(sqn[:], norm_t[:], Act.Sqrt, bias=eps_t[:])
            rec = per.tile([128, T], f32)
            nc.vector.reciprocal(rec[:], sqn[:])
            cos_t = per.tile([128, T], f32)
            nc.vector.tensor_mul(cos_t[:], dot_t[:], rec[:])

            # per-partition top-16 indices (two rounds of top-8)
            mx1 = per.tile([128, 8], f32)
            nc.vector.max(out=mx1[:], in_=cos_t[:])
            ci_t = per.tile([128, 16], u32)
            nc.vector.max_index(ci_t[:, 0:8], mx1[:], cos_t[:])
            cos2 = per.tile([128, T], f32)
            nc.vector.match_replace(
                out=cos2[:], in_to_replace=mx1[:], in_values=cos_t[:],
                imm_value=NEG_FILL,
            )
            mx2 = per.tile([128, 8], f32)
            nc.vector.max(out=mx2[:], in_=cos2[:])
            nc.vector.max_index(ci_t[:, 8:16], mx2[:], cos2[:])

            nc.sync.dma_start(dotb[:], dot_t[:])
            nc.sync.dma_start(normb[:], norm_t[:])
            nc.sync.dma_start(cidx[:], ci_t[:])

    nc.compile()
    return nc


def _get_program():
    if "nc" not in _CACHE:
        _CACHE["nc"] = _build_device_program()
    return _CACHE["nc"]


def _core_starts():
    # core c reads rows [start, start + RP); overlap of RP - RPC = 110 rows
    return [0] + [c * RPC - (RP - RPC) for c in range(1, N_CORES)]


def build_in_maps(embedding, collection):
    emb = np.asarray(embedding, dtype=np.float32)
    coll = np.ascontiguousarray(np.asarray(collection, dtype=np.float32))
    q = emb / np.sqrt(np.sum(emb * emb, dtype=np.float32) + np.float32(1e-12))
    qrep = np.ascontiguousarray(np.broadcast_to(q[None, :], (128, D)))
    in_maps = []
    for start in _core_starts():
        in_maps.append({"coll": coll[start : start + RP], "qrep": qrep})
    return in_maps


def run_device(in_maps, trace=False, **kwargs):
    from concourse.bass_utils import run_bass_kernel_spmd

    nc = _get_program()
    return run_bass_kernel_spmd(
        nc, in_maps, list(range(N_CORES)), trace=trace, **kwargs
    )


def postprocess(results, labels_int):
    labels = np.asarray(labels_int)
    p = np.arange(128, dtype=np.int64)[:, None]
    all_rows, all_vals = [], []
    for c, start in enumerate(_core_starts()):
        dot = results[c]["dotb"]
        norm = results[c]["normb"]
        ci = results[c]["cidx"].astype(np.int64)
        cos = dot / np.sqrt(norm + np.float32(1e-12))
        # tile t = (j, b) at partition p holds shard row j*128*B + p*B + b
        rloc = (ci // B) * (128 * B) + p * B + (ci % B)
        all_rows.append((rloc + start).ravel())
        all_vals.append(cos[p, ci].ravel())
    rows = np.concatenate(all_rows)
    vals = np.concatenate(all_vals)
    # dedupe overlap rows (identical values; keep one)
    rows, uidx = np.unique(rows, return_index=True)
    vals = vals[uidx]
    # global top-(K+1), ties broken by smaller row index (matches lax.top_k)
    order = np.lexsort((rows, -vals))[: K + 1]
    vals11 = vals[order].astype(np.float32)
    idx11 = rows[order]

    neigh = idx11[1:K]
    probs = vals11[1:K]
    preds = labels[neigh].astype(np.int64)
    counts = np.bincount(preds, minlength=NUM_CLASSES)
    pred_int = int(np.argmax(counts))
    conf_pos = int(np.argmax(preds == pred_int))
    confidence = probs[conf_pos]
    return (
        vals11,
        np.int32(pred_int),
        np.float32(confidence),
    )


def kernel(embedding, collection, labels_int):
    in_maps = build_in_maps(embedding, collection)
    res = run_device(in_maps)
    return postprocess(res.results, labels_int)
